# revision 1
# baseline (speedup 1.0000x reference)
"""Causal self-attention Bass kernel for TRN2, 8-core batch x head-group sharding.

Per-core computation (core c: batch b=c//4, head-group g=c%4, 4 heads):
  inputs (pre-transposed, bf16, prepared on host):
    xT   [1024, 2048]  = x[b].T
    wqT  [1024, 256]   = w_q[256g:256g+256, :].T
    wkT  [1024, 256]
    wvT  [1024, 256]
    woT  [256, 1024]   = w_o[:, 256g:256g+256].T
    mask [128, 128]    upper-tri (incl diag) ones, bf16
  output:
    o    [2048, 1024]  f32 partial (host sums 4 partials per batch)

Device layouts:
  qT, kT: [128, 2, 2048] bf16; partition = head-in-pair*64 + d, mid = pair
  v:      [128, 16, 4, 65] bf16; [s%128, s//128, head, d], col 64 = 1.0
  scores: S^T[sk, sq] = matmul(lhsT=kT[64, sk_tile], rhs=qT[64, sq_chunk]);
          two heads row-packed on the PE (rows 0-63 / 64-127), outputs side by
          side in one [128, 1024] psum tile (head i at cols i*512+...)
  P^T = exp(S^T/8) bf16, one ACT op per (t, both heads); diag tiles masked by
          the upper-tri mask (one DVE op over both heads)
  av^T [65, 1024] psum (head i at cols i*512..) accumulated over sk tiles via
          lhsT=[v_h|1]; row 64 = softmax rowsum. Evicted to SBUF f32 fast.
  rowsum: row 64 -> DRAM -> compact [128,8] -> reciprocal -> DRAM ->
          stride-0 broadcast DMA [64, 512] -> TT mult -> attnT bf16
  attnT [128, 2, 2048] bf16; partition = head-in-pair*64 + d
  o psum[sq, dout] = sum over pairs of attnT_pair.T @ woT_pair (K=128 each)
"""

from contextlib import ExitStack

import numpy as np
import ml_dtypes

import concourse.bass as bass
import concourse.mybir as mybir
import concourse.tile as tile

F32 = mybir.dt.float32
BF16 = mybir.dt.bfloat16
AF = mybir.ActivationFunctionType
ALU = mybir.AluOpType

D_MODEL = 1024
N_HEADS = 16
HEAD_DIM = 64
BATCH = 2
SEQ = 2048
N_CORES = 8
HG = 4               # heads per core
DG = HG * HEAD_DIM   # 256 projection dims per core
P = 128
SQB = 512            # sq chunk width
N_SQB = SEQ // SQB   # 4
N_KT = SEQ // P      # 16 sk tiles
N_CT = D_MODEL // P  # 8 contraction tiles
SCALE = 1.0 / np.sqrt(HEAD_DIM)

MAX_WAITS = 1  # this walrus supports a single sync wait per instruction
USE_RECIP_FAST = False


def split_excess_waits(nc):
    """This container's walrus supports 1 sync wait per instruction. Move
    extras onto NoOps inserted immediately before, on the same engine."""
    n_split = 0
    for b in nc.m.functions[0].blocks:
        insts = b.instructions
        i = 0
        while i < len(insts):
            inst = insts[i]
            si = inst.sync_info
            if si is None or si.on_wait is None or len(si.on_wait) <= MAX_WAITS:
                i += 1
                continue
            waits = list(si.on_wait)
            si.on_wait = waits[:MAX_WAITS]
            extra = waits[MAX_WAITS:]
            pos = i
            for j in range(0, len(extra), MAX_WAITS):
                no = mybir.InstNoOp(
                    name=f"{inst.name}_wsplit{n_split}",
                    engine=inst.engine,
                    sync_info=mybir.SyncInfo(
                        on_wait=extra[j : j + MAX_WAITS], on_update=[]
                    ),
                )
                insts.insert(pos, no)
                pos += 1
                n_split += 1
                i += 1
            i += 1
    return n_split


def build_kernel(split_waits=True, repeat=1):
    nc = bass.Bass("TRN2")
    xT = nc.dram_tensor("xT", [D_MODEL, SEQ], BF16, kind="ExternalInput")
    wqT = nc.dram_tensor("wqT", [D_MODEL, DG], BF16, kind="ExternalInput")
    wkT = nc.dram_tensor("wkT", [D_MODEL, DG], BF16, kind="ExternalInput")
    wvT = nc.dram_tensor("wvT", [D_MODEL, DG], BF16, kind="ExternalInput")
    woT = nc.dram_tensor("woT", [DG, D_MODEL], BF16, kind="ExternalInput")
    mask = nc.dram_tensor("mask", [P, P], BF16, kind="ExternalInput")
    o = nc.dram_tensor("o", [SEQ, D_MODEL], F32, kind="ExternalOutput")

    with ExitStack() as ctx:
        tc = ctx.enter_context(tile.TileContext(nc))
        build_body(ctx, tc, xT, wqT, wkT, wvT, woT, mask, o, repeat=repeat)

    if split_waits:
        split_excess_waits(nc)
    return nc


def build_body(ctx, tc, xT, wqT, wkT, wvT, woT, mask, o, repeat=1):
    nc = tc.nc

    consts = ctx.enter_context(tc.tile_pool(name="consts", bufs=1))
    persist = ctx.enter_context(tc.tile_pool(name="persist", bufs=1))
    work = ctx.enter_context(tc.tile_pool(name="work", bufs=6))
    pt_pool = ctx.enter_context(tc.tile_pool(name="pt", bufs=10))
    av_pool = ctx.enter_context(tc.tile_pool(name="avp", bufs=4))
    rs_pool = ctx.enter_context(tc.tile_pool(name="rs", bufs=6))
    dram = ctx.enter_context(tc.tile_pool(name="dram", bufs=3, space="DRAM"))
    # PSUM budget (8 banks of [128, 512]f32):
    #   s (scores, merged heads) [128,1024] x2 bufs = 4 banks
    #   av (merged heads)        [128,1024] x1 buf  = 2 banks
    #   pp (proj A + oproj C)    [128,512]  x2 bufs = 2 banks
    psum_s = ctx.enter_context(tc.tile_pool(name="psum_s", bufs=2, space="PSUM"))
    psum_av = ctx.enter_context(tc.tile_pool(name="psum_av", bufs=1, space="PSUM"))
    psum_p = ctx.enter_context(tc.tile_pool(name="psum_p", bufs=2, space="PSUM"))

    # ---- persistent SBUF tensors
    xT_sb = []
    for ct in range(N_CT):
        xt_t = persist.tile([P, SEQ], BF16, tag=f"xT{ct}", name=f"xT{ct}")
        xT_sb.append(xt_t)
    wqT_sb = persist.tile([P, N_CT, DG], BF16)
    wkT_sb = persist.tile([P, N_CT, DG], BF16)
    wvT_sb = persist.tile([P, N_CT, DG], BF16)
    woT_sb = persist.tile([P, 2, D_MODEL], BF16)
    qT_sb = persist.tile([P, 2, SEQ], BF16)
    kT_sb = persist.tile([P, 2, SEQ], BF16)
    v_sb = persist.tile([P, N_KT, HG, HEAD_DIM + 1], BF16)
    attnT_sb = persist.tile([P, 2, SEQ], BF16)
    trimask = consts.tile([P, P], BF16)

    # ---- input DMAs
    nc.sync.dma_start(wqT_sb[:], wqT.rearrange("(ct p) d -> p ct d", p=P))
    nc.sync.dma_start(wkT_sb[:], wkT.rearrange("(ct p) d -> p ct d", p=P))
    xT3 = xT.rearrange("(ct p) s -> ct p s", p=P)
    for ct in range(N_CT):
        nc.sync.dma_start(xT_sb[ct][:], xT3[ct])
    nc.sync.dma_start(wvT_sb[:], wvT.rearrange("(ct p) d -> p ct d", p=P))
    nc.sync.dma_start(trimask[:], mask[:])
    nc.sync.dma_start(woT_sb[:], woT.rearrange("(ct p) d -> p ct d", p=P))

    # ones column in v (lhsT = [v_h | 1] makes av row 64 the softmax rowsum)
    nc.vector.memset(v_sb[:, :, :, HEAD_DIM], 1.0)

    def proj_qk(w_sb, out_sb, pair, copy_eng, sqbs=None):
        for sqb in (range(N_SQB) if sqbs is None else sqbs):
            ps = psum_p.tile([P, SQB], F32, tag="pp", name="ps_qk")
            for ct in range(N_CT):
                nc.tensor.matmul(
                    ps[:],
                    lhsT=w_sb[:, ct, pair * P : (pair + 1) * P],
                    rhs=xT_sb[ct][:, sqb * SQB : (sqb + 1) * SQB],
                    start=(ct == 0),
                    stop=(ct == N_CT - 1),
                )
            dst = out_sb[:, pair, sqb * SQB : (sqb + 1) * SQB]
            if copy_eng == "act":
                nc.scalar.copy(dst, ps[:])
            else:
                nc.vector.tensor_copy(dst, ps[:])

    def proj_v(sts=None):
        for st in (range(N_KT) if sts is None else sts):
            ps = psum_p.tile([P, SQB], F32, tag="pp", name="ps_v")
            for ct in range(N_CT):
                nc.tensor.matmul(
                    ps[:, 0:DG],
                    lhsT=xT_sb[ct][:, st * P : (st + 1) * P],
                    rhs=wvT_sb[:, ct, :],
                    start=(ct == 0),
                    stop=(ct == N_CT - 1),
                )
            nc.vector.tensor_copy(
                v_sb[:, st, :, 0:HEAD_DIM],
                ps[:, 0:DG].rearrange("p (h d) -> p h d", h=HG),
            )

    def attention_sqb(pair, sqb):
        av = psum_av.tile([P, 2 * SQB], F32, tag="av", name="av")
        for t in range(4 * sqb + 4):
            r = t - 4 * sqb  # >= 0 on the diagonal tile
            off = max(0, r * P)
            w = SQB - off
            sq0 = sqb * SQB + off
            ss = psum_s.tile([P, 2 * SQB], F32, tag="s", name="ss")
            for i in range(2):  # head-in-pair, PE rows i*64..i*64+63
                nc.tensor.matmul(
                    ss[:, i * SQB + off : (i + 1) * SQB],
                    lhsT=kT_sb[i * 64 : (i + 1) * 64, pair, t * P : (t + 1) * P],
                    rhs=qT_sb[i * 64 : (i + 1) * 64, pair, sq0 : sq0 + w],
                    start=True,
                    stop=True,
                )
            pt = pt_pool.tile([P, 2, SQB], BF16, tag="pt", name="pt")
            ss2 = ss[:].rearrange("p (i n) -> p i n", i=2)
            nc.scalar.activation(
                pt[:, :, off:SQB], ss2[:, :, off:SQB], AF.Exp, scale=SCALE
            )
            if r >= 0:
                nc.vector.tensor_tensor(
                    pt[:, :, off : off + P],
                    pt[:, :, off : off + P],
                    trimask[:, None, :].to_broadcast([P, 2, P]),
                    ALU.mult,
                )
            for i in range(2):
                h = 2 * pair + i
                nc.tensor.matmul(
                    av[0 : HEAD_DIM + 1, i * SQB + off : (i + 1) * SQB],
                    lhsT=v_sb[:, t, h, :],
                    rhs=pt[:, i, off:SQB],
                    start=(t == 0),
                    stop=(t == 4 * sqb + 3),
                    skip_group_check=True,
                )
        # evict av (incl rowsum row 64) to SBUF, freeing the psum bank
        av_sb = av_pool.tile([HEAD_DIM + 1, 2 * SQB], F32, tag="avsb", name="av_sb")
        nc.vector.tensor_copy(av_sb[:], av[0 : HEAD_DIM + 1, :])
        # rowsum -> reciprocal -> partition broadcast via DRAM roundtrip
        rs_row = rs_pool.tile([1, 2 * SQB], F32, tag="rsrow", name="rs_row")
        rsdi = dram.tile([1, 2 * SQB], F32, tag="rsdi", name="rsdi")
        rs_b = rs_pool.tile([64, 2 * SQB], F32, tag="rsb", name="rs_b")
        if USE_RECIP_FAST:
            nc.vector.reciprocal_approx_fast(
                rs_row[0:1, :], av_sb[HEAD_DIM : HEAD_DIM + 1, :]
            )
        else:
            nc.vector.tensor_copy(rs_row[0:1, :], av_sb[HEAD_DIM : HEAD_DIM + 1, :])
            nc.vector.reciprocal(rs_row[0:1, :], rs_row[0:1, :])
        nc.sync.dma_start(rsdi[:], rs_row[0:1, :])
        nc.sync.dma_start(rs_b[:], rsdi[0, None, :].to_broadcast([64, 2 * SQB]))
        for i in range(2):
            nc.vector.tensor_tensor(
                attnT_sb[i * 64 : (i + 1) * 64, pair, sqb * SQB : (sqb + 1) * SQB],
                av_sb[0:HEAD_DIM, i * SQB : (i + 1) * SQB],
                rs_b[:, i * SQB : (i + 1) * SQB],
                ALU.mult,
            )

    def oproj(qts):
        for qt in qts:
            for dc in range(2):
                ps = psum_p.tile([P, SQB], F32, tag="pp", name="ps_o")
                for pair in range(2):
                    nc.tensor.matmul(
                        ps[:],
                        lhsT=attnT_sb[:, pair, qt * P : (qt + 1) * P],
                        rhs=woT_sb[:, pair, dc * SQB : (dc + 1) * SQB],
                        start=(pair == 0),
                        stop=(pair == 1),
                    )
                ob = work.tile([P, SQB], F32, tag="ob", name="ob")
                if (qt + dc) % 2 == 0:
                    nc.scalar.copy(ob[:], ps[:])
                else:
                    nc.vector.tensor_copy(ob[:], ps[:])
                nc.sync.dma_start(
                    o[qt * P : (qt + 1) * P, dc * SQB : (dc + 1) * SQB], ob[:]
                )

    # emission order chosen for cross-phase overlap: q/k first (start as soon
    # as the first xT chunk lands), pair-0 attention overlaps pair-1 proj,
    # pair-1 attention overlaps the output projection (per-sqb interleave)
    for _rep in range(repeat):
        proj_qk(wqT_sb, qT_sb, 0, "vec")
        proj_qk(wkT_sb, kT_sb, 0, "vec")
        proj_v()
        seq = list(range(N_SQB - 1, -1, -1))
        for idx, sqb in enumerate(seq):
            attention_sqb(0, sqb)
            proj_qk(wqT_sb, qT_sb, 1, "vec", sqbs=[seq[idx]])
            proj_qk(wkT_sb, kT_sb, 1, "vec", sqbs=[seq[idx]])
        for idx, sqb in enumerate(seq):
            attention_sqb(1, sqb)
            if idx >= 1:
                prev = seq[idx - 1]
                oproj(range(4 * prev, 4 * prev + 4))
        oproj(range(4 * seq[-1], 4 * seq[-1] + 4))


def make_trimask():
    return np.triu(np.ones((P, P), np.float32)).astype(ml_dtypes.bfloat16)


def prep_core_inputs(x, w_q, w_k, w_v, w_o):
    """Host-side sharding: returns list of 8 in_maps (bf16, pre-transposed)."""
    bf = ml_dtypes.bfloat16
    x = np.asarray(x, np.float32)
    w_q = np.asarray(w_q, np.float32)
    w_k = np.asarray(w_k, np.float32)
    w_v = np.asarray(w_v, np.float32)
    w_o = np.asarray(w_o, np.float32)
    tri = make_trimask()
    ins = []
    for c in range(N_CORES):
        b, g = divmod(c, HG)
        sl = slice(g * DG, (g + 1) * DG)
        ins.append(
            {
                "xT": np.ascontiguousarray(x[b].T).astype(bf),
                "wqT": np.ascontiguousarray(w_q[sl, :].T).astype(bf),
                "wkT": np.ascontiguousarray(w_k[sl, :].T).astype(bf),
                "wvT": np.ascontiguousarray(w_v[sl, :].T).astype(bf),
                "woT": np.ascontiguousarray(w_o[:, sl].T).astype(bf),
                "mask": tri,
            }
        )
    return ins


def combine_outputs(results):
    """results: list of 8 dicts with 'o' [SEQ, D_MODEL] f32 -> [B, SEQ, D] f32."""
    out = np.zeros((BATCH, SEQ, D_MODEL), np.float32)
    for c, r in enumerate(results):
        out[c // HG] += r["o"]
    return out


_NC_CACHE = None


def _get_nc():
    global _NC_CACHE
    if _NC_CACHE is None:
        _NC_CACHE = build_kernel()
    return _NC_CACHE


def kernel(x, w_q, w_k, w_v, w_o):
    """Full-input entry point: shards across 8 NeuronCores, returns full output."""
    from concourse.bass_utils import run_bass_kernel_spmd

    nc = _get_nc()
    in_maps = prep_core_inputs(x, w_q, w_k, w_v, w_o)
    res = run_bass_kernel_spmd(nc, in_maps, core_ids=list(range(N_CORES)))
    return combine_outputs(res.results)

